# revision 9
# baseline (speedup 1.0000x reference)
"""Fused masked-attention kernel for Trainium2, data-parallel over batch on 8 cores.

Per core (one batch element): computes
  Q = query @ WQ.T ; K = key @ WK.T ; V = value @ WV.T      (H=64)
  S^T[k,q] = (K Q^T)[k,q]  (scores transposed, k on partitions)
  masked -> exp(S*0.125) -> P^T[k,q]
  O_ext^T[h,q] = sum_k V_ext[k,h] P^T[k,q]   (V_ext has a ones column -> row 64 = Z)
  out[q,h] = O^T[h,q] / Z[q]   (via PE transpose + per-partition scalar mult)

Inputs are host-transposed (qT/kT/vT [E,L], maskT [Lk,Lq]) so all on-chip
matmuls have their contraction dim on partitions with zero on-chip transposes
of large tensors. f32 inputs are cast to bf16 during the (SWDGE) DMA so every
matmul runs at full PE rate (fp32 matmul is half-rate LOW_HIGH); accumulation
stays f32 in PSUM and the softmax normalization stays f32.
"""

import numpy as np

import concourse.bass as bass
import concourse.tile as tile
from concourse import bacc, mybir
from concourse import bass_utils

B, L, E, H = 8, 4096, 1024, 64
NCORES = 8
F32 = mybir.dt.float32
BF16 = mybir.dt.bfloat16
U8 = mybir.dt.uint8

LB = 512          # l-block (free dim) for phase 1 and q-block for phase 2
NEG = -30000.0    # masked score value; exp(NEG*0.125) == 0.0 exactly in f32


def build_nc():
    nc = bacc.Bacc(
        "TRN2",
        target_bir_lowering=False,
        debug=False,
        enable_asserts=False,
        num_devices=NCORES,
    )
    qT = nc.dram_tensor("qT", [E, L], F32, kind="ExternalInput").ap()
    kT = nc.dram_tensor("kT", [E, L], F32, kind="ExternalInput").ap()
    vT = nc.dram_tensor("vT", [E, L], F32, kind="ExternalInput").ap()
    maskT = nc.dram_tensor("maskT", [L, L], U8, kind="ExternalInput").ap()
    wqT = nc.dram_tensor("wqT", [E, H], F32, kind="ExternalInput").ap()
    wkT = nc.dram_tensor("wkT", [E, H], F32, kind="ExternalInput").ap()
    wvT = nc.dram_tensor("wvT", [E, H], F32, kind="ExternalInput").ap()
    ident = nc.dram_tensor("ident", [128, 128], F32, kind="ExternalInput").ap()
    out = nc.dram_tensor("out", [L, H], F32, kind="ExternalOutput").ap()

    EXP = mybir.ActivationFunctionType.Exp

    with tile.TileContext(nc) as tc:
        with (
            tc.tile_pool(name="const", bufs=1) as constp,
            tc.tile_pool(name="persist", bufs=1) as persist,
        ):
            ident_sb = constp.tile([128, 128], F32)
            nc.sync.dma_start(ident_sb[:], ident)
            neg_sb = constp.tile([128, LB], F32)
            nc.vector.memset(neg_sb[:], NEG)
            # weights, e-chunked: [128, 8, 64], cast to bf16 during DMA
            wq_sb = constp.tile([128, 8, H], BF16)
            wk_sb = constp.tile([128, 8, H], BF16)
            wv_sb = constp.tile([128, 8, H], BF16)
            nc.gpsimd.dma_start(wq_sb[:], wqT.rearrange("(c p) h -> p c h", p=128))
            nc.gpsimd.dma_start(wk_sb[:], wkT.rearrange("(c p) h -> p c h", p=128))
            nc.gpsimd.dma_start(wv_sb[:], wvT.rearrange("(c p) h -> p c h", p=128))

            # Q^T/K^T [h, l] duplicated in both partition halves for row-packed
            # (tile_position) score matmuls: rows 0-63 == rows 64-127.
            QT_sb = persist.tile([128, L], BF16)
            KT_sb = persist.tile([128, L], BF16)
            V_sb = persist.tile([128, 32, H + 1], BF16)  # V [k, h] + ones col
            nc.vector.memset(V_sb[:, :, H : H + 1], 1.0)

            # e-chunked DRAM views for per-l-block cast loads
            qT_r = qT.rearrange("(c p) l -> p c l", p=128)
            kT_r = kT.rearrange("(c p) l -> p c l", p=128)
            vT_r = vT.rearrange("(c p) l -> p c l", p=128)

            # ---------------- Phase 1: projections ----------------
            with (
                tc.tile_pool(name="qin", bufs=3) as qinp,
                tc.tile_pool(name="kin", bufs=3) as kinp,
                tc.tile_pool(name="vin", bufs=3) as vinp,
                tc.tile_pool(name="ps_qk", bufs=2, space="PSUM") as ps_qk,
                tc.tile_pool(name="ps_v", bufs=2, space="PSUM") as ps_v,
            ):
                for lb in range(L // LB):
                    ls = lb * LB
                    q_in = qinp.tile([128, 8, LB], BF16, tag="qin")
                    k_in = kinp.tile([128, 8, LB], BF16, tag="kin")
                    v_in = vinp.tile([128, 8, LB], BF16, tag="vin")
                    nc.gpsimd.dma_start(q_in[:], qT_r[:, :, ls : ls + LB])
                    nc.gpsimd.dma_start(k_in[:], kT_r[:, :, ls : ls + LB])
                    nc.gpsimd.dma_start(v_in[:], vT_r[:, :, ls : ls + LB])
                    p_qt = ps_qk.tile([64, LB], F32)
                    p_kt = ps_qk.tile([64, LB], F32)
                    for ec in range(8):
                        st, sp = ec == 0, ec == 7
                        nc.tensor.matmul(
                            p_qt[:], wq_sb[:, ec, :], q_in[:, ec, :],
                            start=st, stop=sp,
                        )
                        nc.tensor.matmul(
                            p_kt[:], wk_sb[:, ec, :], k_in[:, ec, :],
                            start=st, stop=sp,
                        )
                    nc.scalar.copy(QT_sb[0:64, ls : ls + LB], p_qt[:])
                    nc.scalar.copy(KT_sb[0:64, ls : ls + LB], p_kt[:])
                    # V: [k,h] layout -> stationary = vT chunk, moving = wvT chunk
                    for sub in range(LB // 128):
                        p_v = ps_v.tile([128, H], F32)
                        for ec in range(8):
                            nc.tensor.matmul(
                                p_v[:],
                                v_in[:, ec, sub * 128 : (sub + 1) * 128],
                                wv_sb[:, ec, :],
                                start=(ec == 0),
                                stop=(ec == 7),
                            )
                        nc.scalar.copy(V_sb[:, lb * 4 + sub, 0:H], p_v[:])

            # duplicate Q^T/K^T into the upper partition half (SBUF->SBUF DMA;
            # compute engines cannot move data across partitions)
            nc.sync.dma_start(QT_sb[64:128, :], QT_sb[0:64, :])
            nc.sync.dma_start(KT_sb[64:128, :], KT_sb[0:64, :])

            # ---------------- Phase 2: scores/softmax/AV ----------------
            maskT_r = maskT.rearrange("(c p) q -> p c q", p=128)
            with (
                tc.tile_pool(name="mask", bufs=2) as mpool,
                tc.tile_pool(name="pt", bufs=4) as ptpool,
                tc.tile_pool(name="osb", bufs=2) as opool,
                tc.tile_pool(name="zinv", bufs=4) as zpool,
                tc.tile_pool(name="otile", bufs=4) as otpool,
                tc.tile_pool(name="ps_st", bufs=4, space="PSUM") as ps_st,
                tc.tile_pool(name="ps_o", bufs=2, space="PSUM") as ps_o,
                tc.tile_pool(name="ps_t", bufs=2, space="PSUM") as ps_t,
            ):
                for qb in range(L // LB):
                    qs = qb * LB
                    mtile = mpool.tile([128, 32, LB], U8)
                    nc.sync.dma_start(mtile[:], maskT_r[:, :, qs : qs + LB])
                    p_o = ps_o.tile([H + 1, LB], F32)
                    for kcp in range(16):
                        kc0, kc1 = 2 * kcp, 2 * kcp + 1
                        p_st0 = ps_st.tile([128, LB], F32, tag="p_st")
                        p_st1 = ps_st.tile([128, LB], F32, tag="p_st")
                        # row-packed pair: rows 0-63 compute kc0, rows 64-127 kc1
                        nc.tensor.matmul(
                            p_st0[:],
                            KT_sb[0:64, kc0 * 128 : (kc0 + 1) * 128],
                            QT_sb[0:64, qs : qs + LB],
                            start=True,
                            stop=True,
                            tile_position=(0, 0),
                        )
                        nc.tensor.matmul(
                            p_st1[:],
                            KT_sb[64:128, kc1 * 128 : (kc1 + 1) * 128],
                            QT_sb[64:128, qs : qs + LB],
                            start=True,
                            stop=True,
                            tile_position=(64, 0),
                        )
                        for kc, p_st in ((kc0, p_st0), (kc1, p_st1)):
                            nc.vector.copy_predicated(p_st[:], mtile[:, kc, :], neg_sb[:])
                            pt = ptpool.tile([128, LB], BF16, tag="pt")
                            nc.scalar.activation(pt[:], p_st[:], EXP, scale=0.125)
                            nc.tensor.matmul(
                                p_o[:], V_sb[:, kc, :], pt[:],
                                start=(kc == 0), stop=(kc == 31),
                            )
                    o_sb = opool.tile([H + 1, LB], F32)
                    nc.scalar.copy(o_sb[:], p_o[:])
                    for sub in range(LB // 128):
                        p_t = ps_t.tile([128, H + 1], F32)
                        nc.tensor.transpose(
                            p_t[:],
                            o_sb[:, sub * 128 : (sub + 1) * 128],
                            ident_sb[0 : H + 1, 0 : H + 1],
                        )
                        zinv = zpool.tile([128, 1], F32)
                        nc.vector.reciprocal(zinv[:], p_t[:, H : H + 1])
                        ot = otpool.tile([128, H], F32)
                        nc.vector.tensor_scalar_mul(ot[:], p_t[:, 0:H], zinv[:])
                        r0 = qs + sub * 128
                        nc.sync.dma_start(out[r0 : r0 + 128, :], ot[:])
    nc.compile()
    return nc


_NC_CACHE = {}


def kernel(query, key, value, mask, WQ, WK, WV):
    if "nc" not in _NC_CACHE:
        _NC_CACHE["nc"] = build_nc()
    nc = _NC_CACHE["nc"]

    ident = np.eye(128, dtype=np.float32)
    wqT = np.ascontiguousarray(np.asarray(WQ, dtype=np.float32).T)
    wkT = np.ascontiguousarray(np.asarray(WK, dtype=np.float32).T)
    wvT = np.ascontiguousarray(np.asarray(WV, dtype=np.float32).T)
    in_maps = []
    for b in range(B):
        in_maps.append(
            {
                "qT": np.ascontiguousarray(np.asarray(query[b], dtype=np.float32).T),
                "kT": np.ascontiguousarray(np.asarray(key[b], dtype=np.float32).T),
                "vT": np.ascontiguousarray(np.asarray(value[b], dtype=np.float32).T),
                "maskT": np.ascontiguousarray(np.asarray(mask[b]).T).view(np.uint8),
                "wqT": wqT,
                "wkT": wkT,
                "wvT": wvT,
                "ident": ident,
            }
        )
    res = bass_utils.run_bass_kernel_spmd(nc, in_maps, core_ids=list(range(NCORES)))
    out = np.stack([res.results[b]["out"] for b in range(B)], axis=0)
    return out


if __name__ == "__main__":
    rng = np.random.default_rng(0)
    q = rng.standard_normal((B, L, E), dtype=np.float32)
    k = rng.standard_normal((B, L, E), dtype=np.float32)
    v = rng.standard_normal((B, L, E), dtype=np.float32)
    m = rng.integers(0, 2, size=(B, L, L)).astype(bool)
    s = 1.0 / np.sqrt(E)
    wq = rng.uniform(-s, s, size=(H, E)).astype(np.float32)
    wk = rng.uniform(-s, s, size=(H, E)).astype(np.float32)
    wv = rng.uniform(-s, s, size=(H, E)).astype(np.float32)
    o = kernel(query=q, key=k, value=v, mask=m, WQ=wq, WK=wk, WV=wv)
    print(o.shape, o.dtype)
